# revision 1
# baseline (speedup 1.0000x reference)
"""CrossAttention Trainium2 kernel.

Data-parallel over batch across 8 NeuronCores (4 batches each).
Host-side prep casts to bf16 and pre-transposes kv/q/weights so every
on-device matmul has its contraction dim on partitions; softmax skips
max-subtraction (logits are bounded ~|6|) and folds the additive mask in
multiplicatively via a host-precomputed exp(mask).
"""
import os
import sys

sys.path.insert(0, "/opt/trn_rl_repo")

VARIANT = os.environ.get("KERNEL_VARIANT", "")

import numpy as np
import ml_dtypes

import concourse.bacc as bacc
import concourse.mybir as mybir
import concourse.tile as tile

BF = ml_dtypes.bfloat16

B, QN, N, DIM, HEADS, HD = 32, 128, 4096, 512, 8, 64
SCALE = HD ** -0.5
NCORES = 8
BL = B // NCORES  # batches per core
NT = N // 128     # 32 token tiles
NCH = 4           # n-chunks per head for QK/exp (1024 wide)
CHW = N // NCH    # 1024

f32 = mybir.dt.float32
bf16 = mybir.dt.bfloat16
MULT = mybir.AluOpType.mult
EXP = mybir.ActivationFunctionType.Exp

_built = {}
_runner = {}


def _emit(nc, reps=1):
    kvT_d = nc.dram_tensor("kvT", [BL, 4, 128, N], bf16, kind="ExternalInput").ap()
    qT_d = nc.dram_tensor("qT", [4, 128, BL * QN], bf16, kind="ExternalInput").ap()
    em_d = nc.dram_tensor("em", [BL, QN, N], bf16, kind="ExternalInput").ap()
    wkvT_d = nc.dram_tensor("wkvT", [4, 128, 2 * DIM], bf16, kind="ExternalInput").ap()
    wqT_d = nc.dram_tensor("wqT", [4, 128, DIM], bf16, kind="ExternalInput").ap()
    wpT_d = nc.dram_tensor("wpT", [4, 128, DIM], bf16, kind="ExternalInput").ap()
    bias_d = nc.dram_tensor("biasb", [128, DIM], f32, kind="ExternalInput").ap()
    out_d = nc.dram_tensor("out", [BL, QN, DIM], f32, kind="ExternalOutput").ap()

    with tile.TileContext(nc) as tc:
        with (
            tc.tile_pool(name="wpool", bufs=1) as wpool,
            tc.tile_pool(name="kvtp", bufs=4) as kvtp,
            tc.tile_pool(name="ktp", bufs=4) as ktp,
            tc.tile_pool(name="vp", bufs=44) as vp,
            tc.tile_pool(name="pp", bufs=2) as pp,
            tc.tile_pool(name="ptp", bufs=2) as ptp,
            tc.tile_pool(name="emp", bufs=2) as emp,
            tc.tile_pool(name="xp", bufs=8) as xp,
            tc.tile_pool(name="outp", bufs=2) as outp,
            tc.tile_pool(name="smallp", bufs=8) as smallp,
            tc.tile_pool(name="mm512", bufs=2, space="PSUM") as mm512,
            tc.tile_pool(name="qkps", bufs=1, space="PSUM") as qkps,
            tc.tile_pool(name="avps", bufs=2, space="PSUM") as avps,
        ):
            # ---- persistent weights ----
            wkvT = []
            wqT = []
            wpT = []
            qT = []
            for t in range(4):
                wk = wpool.tile([128, 2 * DIM], bf16, name=f"wkvT{t}")
                nc.sync.dma_start(out=wk, in_=wkvT_d[t])
                wkvT.append(wk)
                wq = wpool.tile([128, DIM], bf16, name=f"wqT{t}")
                nc.sync.dma_start(out=wq, in_=wqT_d[t])
                wqT.append(wq)
                wp = wpool.tile([128, DIM], bf16, name=f"wpT{t}")
                nc.sync.dma_start(out=wp, in_=wpT_d[t])
                wpT.append(wp)
                qt = wpool.tile([128, BL * QN], bf16, name=f"qT{t}")
                nc.sync.dma_start(out=qt, in_=qT_d[t])
                qT.append(qt)
            bias_sb = wpool.tile([128, DIM], f32, name="bias_sb")
            nc.sync.dma_start(out=bias_sb, in_=bias_d)

            # ---- q projection for all local batches: qhT[co] = [c_out 128, (b q) 512]
            qhT = []
            for co in range(4):
                ps_q = mm512.tile([128, BL * QN], f32, name="ps_mm512")
                for ci in range(4):
                    nc.tensor.matmul(
                        ps_q,
                        wqT[ci][:, co * 128:(co + 1) * 128],
                        qT[ci],
                        start=(ci == 0),
                        stop=(ci == 3),
                    )
                qh = wpool.tile([128, BL * QN], bf16, name=f"qhT{co}")
                nc.vector.tensor_copy(qh, ps_q)
                qhT.append(qh)

            for rep in range(reps):
              for b in range(BL):
                  # ---- load kvT (features x tokens) ----
                  kvt = []
                  for t in range(4):
                      kv_t = kvtp.tile([128, N], bf16, name="kv_t")
                      nc.gpsimd.dma_start(out=kv_t, in_=kvT_d[b, t])
                      kvt.append(kv_t)
                  em_t = emp.tile([128, N], bf16, name="em_t")
                  nc.gpsimd.dma_start(out=em_t, in_=em_d[b])

                  # ---- k projection, feature-major: kt[ko] = [k_out 128, n 4096]
                  kt = []
                  for ko in range(4):
                      k_t = ktp.tile([128, N], bf16, name="k_t")
                      for ch in range(8):
                          ps_k = mm512.tile([128, 512], f32, name="ps_mm512")
                          for ci in range(4):
                              nc.tensor.matmul(
                                  ps_k,
                                  wkvT[ci][:, ko * 128:(ko + 1) * 128],
                                  kvt[ci][:, ch * 512:(ch + 1) * 512],
                                  start=(ci == 0),
                                  stop=(ci == 3),
                              )
                          nc.vector.tensor_copy(k_t[:, ch * 512:(ch + 1) * 512], ps_k)
                      kt.append(k_t)

                  # ---- v projection, token-major: v[tt] = [n 128, v_feat 512]
                  v = []
                  for tt in range(NT):
                      ps_v = mm512.tile([128, 512], f32, name="ps_mm512")
                      for ci in range(4):
                          nc.tensor.matmul(
                              ps_v,
                              kvt[ci][:, tt * 128:(tt + 1) * 128],
                              wkvT[ci][:, DIM:2 * DIM],
                              start=(ci == 0),
                              stop=(ci == 3),
                          )
                      v_t = vp.tile([128, 512], bf16, name="v_t")
                      nc.vector.tensor_copy(v_t, ps_v)
                      v.append(v_t)

                  # ---- attention per head pair ----
                  if VARIANT == "kvponly":
                      ps_o = mm512.tile([128, DIM], f32, name="ps_mm512")
                      nc.tensor.matmul(ps_o, kt[0][:, 0:128], wpT[0], start=True, stop=True)
                      out_sb = outp.tile([128, DIM], f32, name="out_sb")
                      nc.vector.tensor_add(out_sb, ps_o, bias_sb)
                      nc.gpsimd.dma_start(out=out_d[b], in_=out_sb)
                      continue
                  xT = []
                  for pr in range(4):
                      # QK for both heads (row-packed K=64) + exp per chunk
                      p_sb = []
                      for hh in range(2):
                          p_h = pp.tile([128, N], bf16, name="p_h")
                          p_sb.append(p_h)
                      for ch in range(NCH):
                          ps_s0 = qkps.tile([128, CHW], f32, name="ps_s0")
                          ps_s1 = qkps.tile([128, CHW], f32, name="ps_s1")
                          for half in range(CHW // 512):
                              n0 = ch * CHW + half * 512
                              nc.tensor.matmul(
                                  ps_s0[:, half * 512:(half + 1) * 512],
                                  qhT[pr][0:64, b * QN:(b + 1) * QN],
                                  kt[pr][0:64, n0:n0 + 512],
                                  start=True,
                                  stop=True,
                                  tile_position=(0, 0),
                              )
                              nc.tensor.matmul(
                                  ps_s1[:, half * 512:(half + 1) * 512],
                                  qhT[pr][64:128, b * QN:(b + 1) * QN],
                                  kt[pr][64:128, n0:n0 + 512],
                                  start=True,
                                  stop=True,
                                  tile_position=(64, 0),
                              )
                          nc.scalar.activation(
                              p_sb[0][:, ch * CHW:(ch + 1) * CHW], ps_s0, EXP
                          )
                          nc.scalar.activation(
                              p_sb[1][:, ch * CHW:(ch + 1) * CHW], ps_s1, EXP
                          )

                      # mask-multiply + rowsum + normalize + transpose per head
                      pt_sb = []
                      for hh in range(2):
                          rowsum = smallp.tile([128, 1], f32, name="rowsum")
                          nc.vector.scalar_tensor_tensor(
                              out=p_sb[hh],
                              in0=p_sb[hh],
                              scalar=1.0,
                              in1=em_t,
                              op0=MULT,
                              op1=MULT,
                              accum_out=rowsum,
                          )
                          recip = smallp.tile([128, 1], f32, name="recip")
                          nc.vector.reciprocal(recip, rowsum)
                          nc.vector.tensor_scalar_mul(p_sb[hh], p_sb[hh], recip)
                          pt_h = ptp.tile([128, NT, 128], bf16, name="pt_h")
                          nc.sync.dma_start_transpose(pt_h, p_sb[hh])
                          pt_sb.append(pt_h)

                      # AV, column-tiled across the 2 heads
                      ps_x = avps.tile([128, QN], f32, name="ps_x")
                      for i in range(NT):
                          nc.tensor.matmul(
                              ps_x[0:64, :],
                              v[i][:, (2 * pr) * 64:(2 * pr + 1) * 64],
                              pt_sb[0][:, i, :],
                              start=(i == 0),
                              stop=(i == NT - 1),
                              tile_position=(0, 0),
                              skip_group_check=True,
                          )
                          nc.tensor.matmul(
                              ps_x[64:128, :],
                              v[i][:, (2 * pr + 1) * 64:(2 * pr + 2) * 64],
                              pt_sb[1][:, i, :],
                              start=(i == 0),
                              stop=(i == NT - 1),
                              tile_position=(0, 64),
                              skip_group_check=True,
                          )
                      x_t = xp.tile([128, QN], bf16, name="x_t")
                      nc.vector.tensor_copy(x_t, ps_x)
                      xT.append(x_t)

                  # ---- output projection: out[q, o] = sum_c xT[c,q]^T W^T[c,o]
                  ps_o = mm512.tile([128, DIM], f32, name="ps_mm512")
                  for pr in range(4):
                      nc.tensor.matmul(
                          ps_o, xT[pr], wpT[pr], start=(pr == 0), stop=(pr == 3)
                      )
                  out_sb = outp.tile([128, DIM], f32, name="out_sb")
                  nc.vector.tensor_add(out_sb, ps_o, bias_sb)
                  nc.gpsimd.dma_start(out=out_d[b], in_=out_sb)
    return nc


def build(reps=1):
    if reps not in _built:
        nc = bacc.Bacc(
            "TRN2", target_bir_lowering=False, debug=False, num_devices=NCORES
        )
        _emit(nc, reps)
        nc.compile()
        _built[reps] = nc
    return _built[reps]


def prep_inputs(q, kv, key_mask, Wq, Wkv, Wproj, bproj):
    """Host-side shard + layout prep. Returns per-core in_maps."""
    q = np.asarray(q, dtype=np.float32)
    kv = np.asarray(kv, dtype=np.float32)
    key_mask = np.asarray(key_mask, dtype=np.float32)
    wkvT = np.ascontiguousarray(np.asarray(Wkv, np.float32).T).astype(BF)
    wkvT = wkvT.reshape(4, 128, 2 * DIM)
    wqT = np.ascontiguousarray((np.asarray(Wq, np.float32) * SCALE).T).astype(BF)
    wqT = wqT.reshape(4, 128, DIM)
    wpT = np.ascontiguousarray(np.asarray(Wproj, np.float32).T).astype(BF)
    wpT = wpT.reshape(4, 128, DIM)
    biasb = np.ascontiguousarray(
        np.broadcast_to(np.asarray(bproj, np.float32), (128, DIM))
    )

    kv_bf = kv.astype(BF)
    em = np.exp(key_mask).astype(BF)

    in_maps = []
    for c in range(NCORES):
        sl = slice(c * BL, (c + 1) * BL)
        kvT = np.ascontiguousarray(kv_bf[sl].transpose(0, 2, 1)).reshape(
            BL, 4, 128, N
        )
        q_loc = q[sl].astype(BF)  # [BL, QN, DIM]
        qT = np.ascontiguousarray(q_loc.transpose(2, 0, 1)).reshape(4, 128, BL * QN)
        in_maps.append(
            {
                "kvT": kvT,
                "qT": qT,
                "em": np.ascontiguousarray(em[sl]),
                "wkvT": wkvT,
                "wqT": wqT,
                "wpT": wpT,
                "biasb": biasb,
            }
        )
    return in_maps


class Runner:
    """Jitted SPMD executor with device-resident inputs for repeat timing."""

    def __init__(self, reps=1):
        import jax
        from concourse.bass2jax import (
            _bass_exec_p,
            install_neuronx_cc_hook,
            partition_id_tensor,
        )
        from jax.experimental.shard_map import shard_map
        from jax.sharding import Mesh, PartitionSpec

        self.jax = jax
        nc = build(reps)
        install_neuronx_cc_hook()
        pname = nc.partition_id_tensor.name if nc.partition_id_tensor else None
        in_names, out_names, out_avals = [], [], []
        for alloc in nc.m.functions[0].allocations:
            if not isinstance(alloc, mybir.MemoryLocationSet):
                continue
            name = alloc.memorylocations[0].name
            if alloc.kind == "ExternalInput":
                if name != pname:
                    in_names.append(name)
            elif alloc.kind == "ExternalOutput":
                out_names.append(name)
                out_avals.append(
                    jax.core.ShapedArray(
                        tuple(alloc.tensor_shape), mybir.dt.np(alloc.dtype)
                    )
                )
        self.in_names = list(in_names)
        self.out_names = out_names
        self.out_avals = out_avals
        n_params = len(in_names)
        all_names = in_names + out_names
        if pname is not None:
            all_names = all_names + [pname]
        donate = tuple(range(n_params, n_params + len(out_names)))

        def _body(*args):
            operands = list(args)
            if pname is not None:
                operands.append(partition_id_tensor())
            outs = _bass_exec_p.bind(
                *operands,
                out_avals=tuple(out_avals),
                in_names=tuple(all_names),
                out_names=tuple(out_names),
                lowering_input_output_aliases=(),
                sim_require_finite=True,
                sim_require_nnan=True,
                nc=nc,
            )
            return tuple(outs)

        devices = jax.devices()[:NCORES]
        self.mesh = Mesh(np.asarray(devices), ("core",))
        self.pspec = PartitionSpec("core")
        in_specs = (self.pspec,) * (n_params + len(out_names))
        out_specs = (self.pspec,) * len(out_names)
        self.fn = jax.jit(
            shard_map(
                _body,
                mesh=self.mesh,
                in_specs=in_specs,
                out_specs=out_specs,
                check_rep=False,
            ),
            donate_argnums=donate,
            keep_unused=True,
        )

    def put_inputs(self, in_maps):
        """Concat per-core inputs on axis 0 and move to devices (sharded)."""
        from jax.sharding import NamedSharding

        sh = NamedSharding(self.mesh, self.pspec)
        dev = []
        for name in self.in_names:
            cat = np.concatenate([m[name] for m in in_maps], axis=0)
            dev.append(self.jax.device_put(cat, sh))
        return dev

    def zeros(self):
        from jax.sharding import NamedSharding

        sh = NamedSharding(self.mesh, self.pspec)
        return [
            self.jax.device_put(
                np.zeros((NCORES * a.shape[0], *a.shape[1:]), a.dtype), sh
            )
            for a in self.out_avals
        ]

    def run(self, dev_inputs, zeros=None):
        if zeros is None:
            zeros = self.zeros()
        outs = self.fn(*dev_inputs, *zeros)
        self.jax.block_until_ready(outs)
        return outs


def get_runner(reps=1):
    if reps not in _runner:
        _runner[reps] = Runner(reps)
    return _runner[reps]


def kernel(q, kv, key_mask, Wq, Wkv, Wproj, bproj):
    r = get_runner()
    in_maps = prep_inputs(q, kv, key_mask, Wq, Wkv, Wproj, bproj)
    dev = r.put_inputs(in_maps)
    outs = r.run(dev)
    out = np.asarray(outs[0]).reshape(NCORES, BL, QN, DIM).reshape(B, QN, DIM)
    return out.astype(np.float32)



# revision 24
# speedup vs baseline: 1.2783x; 1.2783x over previous
"""CrossAttention Trainium2 kernel, v2.

Data-parallel over batch across 8 NeuronCores (4 batches each).

v1 computed attention probabilities P in [query, key] orientation and
transposed them with dma_start_transpose (33.6MB/core of 2-byte-element
XBAR traffic) — that dominated the runtime. v2 computes S^T = K^T·Q in
[key, query] orientation directly, so P^T feeds the AV matmul with no
transpose at all:

  - QK: per 128-token tile, matmul(lhsT=k_tile[hd,128], rhs=qh[hd,128])
    packs the two heads of a pair in PE row-halves (tile_position).
  - softmax: no max-subtraction (logits bounded); exp on ACT engine out
    of PSUM; additive mask folded in multiplicatively (host precomputes
    exp(mask), transposed layout) on DVE.
  - denominators: V gets a ones-column appended (M=65 AV matmuls), so
    row 64 of the AV accumulator is sum_n p — free.
  - normalization: folded into a per-head output projection; denom
    reciprocals land on q-partitions via 8 tiny PE transposes, then one
    fused DVE scalar_tensor_tensor per head does scale+accumulate(+bias).

Engine budget per core (cost model): PE ~335us, ACT ~150us, DVE ~150us,
Pool ~140us, DMA ~60us.
"""
import os
import sys

sys.path.insert(0, "/opt/trn_rl_repo")

VARIANT = os.environ.get("KERNEL_VARIANT", "")

import numpy as np
import ml_dtypes

import concourse.bacc as bacc
import concourse.mybir as mybir
import concourse.tile as tile

BF = ml_dtypes.bfloat16

B, QN, N, DIM, HEADS, HD = 32, 128, 4096, 512, 8, 64
SCALE = HD ** -0.5
NCORES = 8
BL = B // NCORES  # batches per core
NT = N // 128     # 32 token tiles
NG = 4            # QK/exp groups per head (8 tiles = 1024 wide each)
GW = N // NG      # group width (psum free bytes: 4KB = 2 banks)

f32 = mybir.dt.float32
bf16 = mybir.dt.bfloat16
MULT = mybir.AluOpType.mult
ADD = mybir.AluOpType.add
EXP = mybir.ActivationFunctionType.Exp

_built = {}
_runner = {}


def _emit(nc, reps=1):
    kvT_d = nc.dram_tensor("kvT", [BL, 4, 128, N], bf16, kind="ExternalInput").ap()
    qT_d = nc.dram_tensor("qT", [4, 128, BL * QN], bf16, kind="ExternalInput").ap()
    emT_d = nc.dram_tensor("emT", [BL, 128, N], bf16, kind="ExternalInput").ap()
    wkvT_d = nc.dram_tensor("wkvT", [4, 128, 2 * DIM], bf16, kind="ExternalInput").ap()
    wqT_d = nc.dram_tensor("wqT", [4, 128, DIM], bf16, kind="ExternalInput").ap()
    wpT_d = nc.dram_tensor("wpT", [HEADS, 64, DIM], bf16, kind="ExternalInput").ap()
    bias_d = nc.dram_tensor("biasb", [128, DIM], f32, kind="ExternalInput").ap()
    out_d = nc.dram_tensor("out", [BL, QN, DIM], f32, kind="ExternalOutput").ap()
    dbg = os.environ.get("KERNEL_DEBUG", "") == "1"
    if dbg:
        dbg_kt = nc.dram_tensor("dbg_kt", [4, 128, N], bf16, kind="ExternalOutput").ap()
        dbg_pt = nc.dram_tensor("dbg_pt", [2, 128, N], bf16, kind="ExternalOutput").ap()
        dbg_v = nc.dram_tensor("dbg_v", [2, 128, HEADS, 65], bf16, kind="ExternalOutput").ap()
        dbg_x = nc.dram_tensor("dbg_x", [64, HEADS, 128], bf16, kind="ExternalOutput").ap()
        dbg_d = nc.dram_tensor("dbg_d", [1, HEADS, 128], f32, kind="ExternalOutput").ap()
        dbg_r = nc.dram_tensor("dbg_r", [128, HEADS], f32, kind="ExternalOutput").ap()

    with tile.TileContext(nc) as tc:
        with (
            tc.tile_pool(name="wpool", bufs=1) as wpool,
            tc.tile_pool(name="kvtp", bufs=4) as kvtp,
            tc.tile_pool(name="ktp", bufs=4) as ktp,
            tc.tile_pool(name="vp", bufs=NT) as vp,
            tc.tile_pool(name="emp", bufs=2) as emp,
            tc.tile_pool(name="ptp", bufs=2) as ptp,
            tc.tile_pool(name="xsp", bufs=2) as xsp,
            tc.tile_pool(name="mm512", bufs=2, space="PSUM") as mm512,
            tc.tile_pool(name="qkps", bufs=1, space="PSUM") as qkps,
            tc.tile_pool(name="xaps", bufs=1, space="PSUM") as xaps,
        ):
            # ---- persistent weights ----
            wkvT, wqT, wpT, qT = [], [], [], []
            for t in range(4):
                wk = wpool.tile([128, 2 * DIM], bf16, name=f"wkvT{t}")
                nc.sync.dma_start(out=wk, in_=wkvT_d[t])
                wkvT.append(wk)
                wq = wpool.tile([128, DIM], bf16, name=f"wqT{t}")
                nc.sync.dma_start(out=wq, in_=wqT_d[t])
                wqT.append(wq)
                qt = wpool.tile([128, BL * QN], bf16, name=f"qT{t}")
                nc.sync.dma_start(out=qt, in_=qT_d[t])
                qT.append(qt)
            for h in range(HEADS):
                wp = wpool.tile([64, DIM], bf16, name=f"wpT{h}")
                nc.sync.dma_start(out=wp, in_=wpT_d[h])
                wpT.append(wp)
            bias_sb = wpool.tile([128, DIM], f32, name="bias_sb")
            nc.sync.dma_start(out=bias_sb, in_=bias_d)
            ident1 = wpool.tile([1, 1], f32, name="ident1")
            nc.vector.memset(ident1, 1.0)

            # ---- q projection for all local batches: qhT[co] = [c 128, (b q) 512]
            qhT = []
            for co in range(4):
                ps_q = mm512.tile([128, BL * QN], f32, name="ps_mm512")
                for ci in range(4):
                    nc.tensor.matmul(
                        ps_q,
                        wqT[ci][:, co * 128:(co + 1) * 128],
                        qT[ci],
                        start=(ci == 0),
                        stop=(ci == 3),
                    )
                qh = wpool.tile([128, BL * QN], bf16, name=f"qhT{co}")
                nc.vector.tensor_copy(qh, ps_q)
                qhT.append(qh)

            def fetch(b):
                """Issue DMA loads for step with batch b; returns tiles."""
                kvt = []
                for t in range(4):
                    kv_t = kvtp.tile([128, N], bf16, name="kv_t")
                    nc.sync.dma_start(out=kv_t, in_=kvT_d[b, t])
                    kvt.append(kv_t)
                em_t = emp.tile([128, N], bf16, name="em_t")
                nc.sync.dma_start(out=em_t, in_=emT_d[b])
                return kvt, em_t

            def proj_denoms(xaug):
                """Copy X^T + denoms out of PSUM; reciprocals on q-partitions."""
                x_sb = xsp.tile([64, HEADS, 128], bf16, name="x_sb")
                nc.vector.tensor_copy(x_sb, xaug[0:64])
                d_sb = xsp.tile([1, HEADS, 128], f32, name="d_sb")
                nc.vector.tensor_copy(d_sb, xaug[64:65])
                dT = mm512.tile([128, 512], f32, name="ps_mm512")
                for h in range(HEADS):
                    nc.tensor.matmul(
                        dT[:, h:h + 1],
                        d_sb[:, h, :],
                        ident1,
                        is_transpose=True,
                        start=True,
                        stop=True,
                    )
                dtp_sb = xsp.tile([128, HEADS], f32, name="dtp_sb")
                nc.vector.tensor_copy(dtp_sb, dT[:, 0:HEADS])
                recips = xsp.tile([128, HEADS], f32, name="recips")
                nc.vector.reciprocal(recips, dtp_sb)
                return x_sb, recips

            def proj_head(h, x_sb, recips, acc):
                """One head of output projection + fused normalize-accumulate."""
                ps = mm512.tile([128, DIM], f32, name="ps_mm512")
                nc.tensor.matmul(
                    ps,
                    x_sb[:, h, :],
                    wpT[h],
                    start=True,
                    stop=True,
                )
                nc.vector.scalar_tensor_tensor(
                    out=acc,
                    in0=ps,
                    scalar=recips[:, h:h + 1],
                    in1=(bias_sb if h == 0 else acc),
                    op0=MULT,
                    op1=ADD,
                )

            def kquad(kvt, kt, ko, ch):
                ps = mm512.tile([128, 512], f32, name="ps_mm512")
                for ci in range(4):
                    nc.tensor.matmul(
                        ps,
                        wkvT[ci][:, ko * 128:(ko + 1) * 128],
                        kvt[ci][:, ch * 512:(ch + 1) * 512],
                        start=(ci == 0),
                        stop=(ci == 3),
                    )
                nc.vector.tensor_copy(kt[ko][:, ch * 512:(ch + 1) * 512], ps)

            def vquad(kvt, vt, tt):
                ps = mm512.tile([128, 512], f32, name="ps_mm512")
                for ci in range(4):
                    nc.tensor.matmul(
                        ps,
                        kvt[ci][:, tt * 128:(tt + 1) * 128],
                        wkvT[ci][:, DIM:2 * DIM],
                        start=(ci == 0),
                        stop=(ci == 3),
                    )
                nc.scalar.copy(
                    vt[tt][:, :, 0:64], ps[:, :].rearrange("p (h d) -> p h d", h=HEADS)
                )
                nc.gpsimd.memset(vt[tt][:, :, 64:65], 1.0)

            def emit_av_pair(xaug, vt, pr, pt0, pt1):
                # One fully-serial 32-matmul accumulation chain per head:
                # interleaved open chains in one PSUM bank corrupt the
                # accumulator, so heads never interleave.
                for h, pt in ((2 * pr, pt0), (2 * pr + 1, pt1)):
                    for t in range(NT):
                        nc.tensor.matmul(
                            xaug[:, h, :],
                            vt[t][:, h, :],
                            pt[:, t * 128:(t + 1) * 128],
                            start=(t == 0),
                            stop=(t == NT - 1),
                            skip_group_check=True,
                        )

            steps = [b for _ in range(reps) for b in range(BL)]
            fetched = fetch(steps[0])
            pending = None  # (b, x_sb, recips) awaiting proj phase 2

            for i, b in enumerate(steps):
                kvt, em_t = fetched
                # ---- A phase: kv projection (+ dribbled proj of prev batch)
                kt = [ktp.tile([128, N], bf16, name="k_t") for _ in range(4)]
                vt = [vp.tile([128, HEADS, 65], bf16, name="v_t") for _ in range(NT)]
                quads = [("k", ko, ch) for ko in range(4) for ch in range(N // 512)]
                quads += [("v", tt, 0) for tt in range(NT)]
                acc = None
                for qi, (kind, a0, a1) in enumerate(quads):
                    if pending is not None and qi < HEADS:
                        if qi == 0:
                            acc = xsp.tile([128, DIM], f32, name="acc")
                        proj_head(qi, pending[1], pending[2], acc)
                    if kind == "k":
                        kquad(kvt, kt, a0, a1)
                    else:
                        vquad(kvt, vt, a0)
                    if pending is not None and qi == HEADS:
                        nc.sync.dma_start(out=out_d[pending[0]], in_=acc)
                        pending = None

                # ---- B phase: attention, S^T orientation
                xaug = xaps.tile([65, HEADS, 128], f32, name="xaug")
                av_prev = None
                for pr in range(4):
                    pt0 = ptp.tile([128, N], bf16, name="pt0")
                    pt1 = ptp.tile([128, N], bf16, name="pt1")
                    for g in range(NG):
                        ps0 = qkps.tile([128, GW], f32, name="ps_s0")
                        ps1 = qkps.tile([128, GW], f32, name="ps_s1")
                        for j in range(GW // 128):
                            t = (GW // 128) * g + j
                            nc.tensor.matmul(
                                ps0[:, j * 128:(j + 1) * 128],
                                kt[pr][0:64, t * 128:(t + 1) * 128],
                                qhT[pr][0:64, b * QN:(b + 1) * QN],
                                start=True,
                                stop=True,
                                tile_position=(0, 0),
                            )
                            nc.tensor.matmul(
                                ps1[:, j * 128:(j + 1) * 128],
                                kt[pr][64:128, t * 128:(t + 1) * 128],
                                qhT[pr][64:128, b * QN:(b + 1) * QN],
                                start=True,
                                stop=True,
                                tile_position=(64, 0),
                            )
                        sl = slice(g * GW, (g + 1) * GW)
                        nc.scalar.activation(pt0[:, sl], ps0, EXP)
                        nc.scalar.activation(pt1[:, sl], ps1, EXP)
                        nc.vector.tensor_mul(pt0[:, sl], pt0[:, sl], em_t[:, sl])
                        nc.vector.tensor_mul(pt1[:, sl], pt1[:, sl], em_t[:, sl])
                        if pr == 1 and g == 0 and i + 1 < len(steps):
                            fetched = fetch(steps[i + 1])
                    if dbg and i == 0 and pr == 0:
                        nc.sync.dma_start(out=dbg_pt[0], in_=pt0)
                        nc.sync.dma_start(out=dbg_pt[1], in_=pt1)
                    if av_prev is not None:
                        emit_av_pair(xaug, vt, *av_prev)
                    av_prev = (pr, pt0, pt1)
                emit_av_pair(xaug, vt, *av_prev)
                if dbg and i == 0:
                    for t in range(4):
                        nc.sync.dma_start(out=dbg_kt[t], in_=kt[t])
                    for t in range(2):
                        nc.sync.dma_start(out=dbg_v[t], in_=vt[t])

                # ---- proj phase 1: denominators to q-partitions
                x_sb, recips = proj_denoms(xaug)
                if dbg and i == 0:
                    nc.sync.dma_start(out=dbg_x, in_=x_sb)
                    nc.sync.dma_start(out=dbg_r, in_=recips)
                pending = (b, x_sb, recips)

            # epilogue: flush last batch's projection
            acc = xsp.tile([128, DIM], f32, name="acc")
            for h in range(HEADS):
                proj_head(h, pending[1], pending[2], acc)
            nc.sync.dma_start(out=out_d[pending[0]], in_=acc)
    return nc


def build(reps=1):
    if reps not in _built:
        nc = bacc.Bacc(
            "TRN2", target_bir_lowering=False, debug=False, num_devices=NCORES
        )
        _emit(nc, reps)
        nc.compile()
        _built[reps] = nc
    return _built[reps]


def prep_inputs(q, kv, key_mask, Wq, Wkv, Wproj, bproj):
    """Host-side shard + layout prep. Returns per-core in_maps."""
    q = np.asarray(q, dtype=np.float32)
    kv = np.asarray(kv, dtype=np.float32)
    key_mask = np.asarray(key_mask, dtype=np.float32)
    wkvT = np.ascontiguousarray(np.asarray(Wkv, np.float32).T).astype(BF)
    wkvT = wkvT.reshape(4, 128, 2 * DIM)
    wqT = np.ascontiguousarray((np.asarray(Wq, np.float32) * SCALE).T).astype(BF)
    wqT = wqT.reshape(4, 128, DIM)
    wpT = np.ascontiguousarray(np.asarray(Wproj, np.float32).T).astype(BF)
    wpT = wpT.reshape(HEADS, 64, DIM)
    biasb = np.ascontiguousarray(
        np.broadcast_to(np.asarray(bproj, np.float32), (128, DIM))
    )

    kv_bf = kv.astype(BF)
    em = np.exp(key_mask).astype(BF)  # [B, QN, N]

    in_maps = []
    for c in range(NCORES):
        sl = slice(c * BL, (c + 1) * BL)
        kvT = np.ascontiguousarray(kv_bf[sl].transpose(0, 2, 1)).reshape(
            BL, 4, 128, N
        )
        q_loc = q[sl].astype(BF)  # [BL, QN, DIM]
        qT = np.ascontiguousarray(q_loc.transpose(2, 0, 1)).reshape(4, 128, BL * QN)
        # emT[b, n_in_tile, tt*128 + q] = em[b, q, tt*128 + n_in_tile]
        emT = em[sl].reshape(BL, QN, NT, 128).transpose(0, 3, 2, 1)
        emT = np.ascontiguousarray(emT).reshape(BL, 128, N)
        in_maps.append(
            {
                "kvT": kvT,
                "qT": qT,
                "emT": emT,
                "wkvT": wkvT,
                "wqT": wqT,
                "wpT": wpT,
                "biasb": biasb,
            }
        )
    return in_maps


class Runner:
    """Jitted SPMD executor with device-resident inputs for repeat timing."""

    def __init__(self, reps=1):
        import jax
        from concourse.bass2jax import (
            _bass_exec_p,
            install_neuronx_cc_hook,
            partition_id_tensor,
        )
        from jax.experimental.shard_map import shard_map
        from jax.sharding import Mesh, PartitionSpec

        self.jax = jax
        nc = build(reps)
        install_neuronx_cc_hook()
        pname = nc.partition_id_tensor.name if nc.partition_id_tensor else None
        in_names, out_names, out_avals = [], [], []
        for alloc in nc.m.functions[0].allocations:
            if not isinstance(alloc, mybir.MemoryLocationSet):
                continue
            name = alloc.memorylocations[0].name
            if alloc.kind == "ExternalInput":
                if name != pname:
                    in_names.append(name)
            elif alloc.kind == "ExternalOutput":
                out_names.append(name)
                out_avals.append(
                    jax.core.ShapedArray(
                        tuple(alloc.tensor_shape), mybir.dt.np(alloc.dtype)
                    )
                )
        self.in_names = list(in_names)
        self.out_names = out_names
        self.out_avals = out_avals
        n_params = len(in_names)
        all_names = in_names + out_names
        if pname is not None:
            all_names = all_names + [pname]
        donate = tuple(range(n_params, n_params + len(out_names)))

        def _body(*args):
            operands = list(args)
            if pname is not None:
                operands.append(partition_id_tensor())
            outs = _bass_exec_p.bind(
                *operands,
                out_avals=tuple(out_avals),
                in_names=tuple(all_names),
                out_names=tuple(out_names),
                lowering_input_output_aliases=(),
                sim_require_finite=True,
                sim_require_nnan=True,
                nc=nc,
            )
            return tuple(outs)

        devices = jax.devices()[:NCORES]
        self.mesh = Mesh(np.asarray(devices), ("core",))
        self.pspec = PartitionSpec("core")
        in_specs = (self.pspec,) * (n_params + len(out_names))
        out_specs = (self.pspec,) * len(out_names)
        self.fn = jax.jit(
            shard_map(
                _body,
                mesh=self.mesh,
                in_specs=in_specs,
                out_specs=out_specs,
                check_rep=False,
            ),
            donate_argnums=donate,
            keep_unused=True,
        )

    def put_inputs(self, in_maps):
        """Concat per-core inputs on axis 0 and move to devices (sharded)."""
        from jax.sharding import NamedSharding

        sh = NamedSharding(self.mesh, self.pspec)
        dev = []
        for name in self.in_names:
            cat = np.concatenate([m[name] for m in in_maps], axis=0)
            dev.append(self.jax.device_put(cat, sh))
        return dev

    def zeros(self):
        from jax.sharding import NamedSharding

        sh = NamedSharding(self.mesh, self.pspec)
        return [
            self.jax.device_put(
                np.zeros((NCORES * a.shape[0], *a.shape[1:]), a.dtype), sh
            )
            for a in self.out_avals
        ]

    def run(self, dev_inputs, zeros=None):
        if zeros is None:
            zeros = self.zeros()
        outs = self.fn(*dev_inputs, *zeros)
        self.jax.block_until_ready(outs)
        return outs


def get_runner(reps=1):
    if reps not in _runner:
        _runner[reps] = Runner(reps)
    return _runner[reps]


def kernel(q, kv, key_mask, Wq, Wkv, Wproj, bproj):
    r = get_runner()
    in_maps = prep_inputs(q, kv, key_mask, Wq, Wkv, Wproj, bproj)
    dev = r.put_inputs(in_maps)
    outs = r.run(dev)
    out = np.asarray(outs[0]).reshape(NCORES, BL, QN, DIM).reshape(B, QN, DIM)
    return out.astype(np.float32)


# revision 28
# speedup vs baseline: 3.9714x; 3.1068x over previous
"""CrossAttention Trainium2 kernel, v2.

Data-parallel over batch across 8 NeuronCores (4 batches each).

v1 computed attention probabilities P in [query, key] orientation and
transposed them with dma_start_transpose (33.6MB/core of 2-byte-element
XBAR traffic) — that dominated the runtime. v2 computes S^T = K^T·Q in
[key, query] orientation directly, so P^T feeds the AV matmul with no
transpose at all:

  - QK: per 128-token tile, matmul(lhsT=k_tile[hd,128], rhs=qh[hd,128])
    packs the two heads of a pair in PE row-halves (tile_position).
  - softmax: no max-subtraction (logits bounded); exp on ACT engine out
    of PSUM; additive mask folded in multiplicatively (host precomputes
    exp(mask), transposed layout) on DVE.
  - denominators: V gets a ones-column appended (M=65 AV matmuls), so
    row 64 of the AV accumulator is sum_n p — free.
  - normalization: folded into a per-head output projection; denom
    reciprocals land on q-partitions via 8 tiny PE transposes, then one
    fused DVE scalar_tensor_tensor per head does scale+accumulate(+bias).

Engine budget per core (cost model): PE ~335us, ACT ~150us, DVE ~150us,
Pool ~140us, DMA ~60us.
"""
import os
import sys

sys.path.insert(0, "/opt/trn_rl_repo")

VARIANT = os.environ.get("KERNEL_VARIANT", "")

import numpy as np
import ml_dtypes

import concourse.bacc as bacc
import concourse.mybir as mybir
import concourse.tile as tile

BF = ml_dtypes.bfloat16

B, QN, N, DIM, HEADS, HD = 32, 128, 4096, 512, 8, 64
SCALE = HD ** -0.5
NCORES = 8
BL = B // NCORES  # batches per core
NT = N // 128     # 32 token tiles
NG = 4            # QK/exp groups per head (8 tiles = 1024 wide each)
GW = N // NG      # group width (psum free bytes: 4KB = 2 banks)

f32 = mybir.dt.float32
bf16 = mybir.dt.bfloat16
MULT = mybir.AluOpType.mult
ADD = mybir.AluOpType.add
EXP = mybir.ActivationFunctionType.Exp

_built = {}
_runner = {}


def _emit(nc, reps=1):
    kvT_d = nc.dram_tensor("kvT", [BL, 4, 128, N], bf16, kind="ExternalInput").ap()
    qT_d = nc.dram_tensor("qT", [4, 128, BL * QN], bf16, kind="ExternalInput").ap()
    emT_d = nc.dram_tensor("emT", [BL, 128, N], bf16, kind="ExternalInput").ap()
    wkvT_d = nc.dram_tensor("wkvT", [4, 128, 2 * DIM], bf16, kind="ExternalInput").ap()
    wqT_d = nc.dram_tensor("wqT", [4, 128, DIM], bf16, kind="ExternalInput").ap()
    wpT_d = nc.dram_tensor("wpT", [HEADS, 64, DIM], bf16, kind="ExternalInput").ap()
    bias_d = nc.dram_tensor("biasb", [128, DIM], f32, kind="ExternalInput").ap()
    out_d = nc.dram_tensor("out", [BL, QN, DIM], f32, kind="ExternalOutput").ap()
    dbg = os.environ.get("KERNEL_DEBUG", "") == "1"
    if dbg:
        dbg_kt = nc.dram_tensor("dbg_kt", [4, 128, N], bf16, kind="ExternalOutput").ap()
        dbg_pt = nc.dram_tensor("dbg_pt", [2, 128, N], bf16, kind="ExternalOutput").ap()
        dbg_v = nc.dram_tensor("dbg_v", [2, 128, HEADS, 65], bf16, kind="ExternalOutput").ap()
        dbg_x = nc.dram_tensor("dbg_x", [64, HEADS, 128], bf16, kind="ExternalOutput").ap()
        dbg_d = nc.dram_tensor("dbg_d", [1, HEADS, 128], f32, kind="ExternalOutput").ap()
        dbg_r = nc.dram_tensor("dbg_r", [128, HEADS], f32, kind="ExternalOutput").ap()

    with tile.TileContext(nc) as tc:
        with (
            tc.tile_pool(name="wpool", bufs=1) as wpool,
            tc.tile_pool(name="kvtp", bufs=4) as kvtp,
            tc.tile_pool(name="ktp", bufs=4) as ktp,
            tc.tile_pool(name="vp", bufs=NT) as vp,
            tc.tile_pool(name="emp", bufs=2) as emp,
            tc.tile_pool(name="ptp", bufs=2) as ptp,
            tc.tile_pool(name="xsp", bufs=2) as xsp,
            tc.tile_pool(name="mm512", bufs=2, space="PSUM") as mm512,
            tc.tile_pool(name="qkps", bufs=1, space="PSUM") as qkps,
            tc.tile_pool(name="xaps", bufs=1, space="PSUM") as xaps,
        ):
            # ---- persistent weights ----
            wkvT, wqT, wpT, qT = [], [], [], []
            for t in range(4):
                wk = wpool.tile([128, 2 * DIM], bf16, name=f"wkvT{t}")
                nc.sync.dma_start(out=wk, in_=wkvT_d[t])
                wkvT.append(wk)
                wq = wpool.tile([128, DIM], bf16, name=f"wqT{t}")
                nc.sync.dma_start(out=wq, in_=wqT_d[t])
                wqT.append(wq)
                qt = wpool.tile([128, BL * QN], bf16, name=f"qT{t}")
                nc.sync.dma_start(out=qt, in_=qT_d[t])
                qT.append(qt)
            for h in range(HEADS):
                wp = wpool.tile([64, DIM], bf16, name=f"wpT{h}")
                nc.sync.dma_start(out=wp, in_=wpT_d[h])
                wpT.append(wp)
            bias_sb = wpool.tile([128, DIM], f32, name="bias_sb")
            nc.sync.dma_start(out=bias_sb, in_=bias_d)
            ident1 = wpool.tile([1, 1], f32, name="ident1")
            nc.vector.memset(ident1, 1.0)

            # ---- q projection for all local batches: qhT[co] = [c 128, (b q) 512]
            qhT = []
            for co in range(4):
                ps_q = mm512.tile([128, BL * QN], f32, name="ps_mm512")
                for ci in range(4):
                    nc.tensor.matmul(
                        ps_q,
                        wqT[ci][:, co * 128:(co + 1) * 128],
                        qT[ci],
                        start=(ci == 0),
                        stop=(ci == 3),
                    )
                qh = wpool.tile([128, BL * QN], bf16, name=f"qhT{co}")
                nc.vector.tensor_copy(qh, ps_q)
                qhT.append(qh)

            def fetch(b):
                """Issue DMA loads for step with batch b; returns tiles."""
                kvt = []
                for t in range(4):
                    kv_t = kvtp.tile([128, N], bf16, name="kv_t")
                    nc.sync.dma_start(out=kv_t, in_=kvT_d[b, t])
                    kvt.append(kv_t)
                em_t = emp.tile([128, N], bf16, name="em_t")
                nc.sync.dma_start(out=em_t, in_=emT_d[b])
                return kvt, em_t

            def proj_denoms(xaug):
                """Copy X^T + denoms out of PSUM; reciprocals on q-partitions."""
                x_sb = xsp.tile([64, HEADS, 128], bf16, name="x_sb")
                nc.vector.tensor_copy(x_sb, xaug[0:64])
                d_sb = xsp.tile([1, HEADS, 128], f32, name="d_sb")
                nc.vector.tensor_copy(d_sb, xaug[64:65])
                dT = mm512.tile([128, 512], f32, name="ps_mm512")
                for h in range(HEADS):
                    nc.tensor.matmul(
                        dT[:, h:h + 1],
                        d_sb[:, h, :],
                        ident1,
                        is_transpose=True,
                        start=True,
                        stop=True,
                    )
                dtp_sb = xsp.tile([128, HEADS], f32, name="dtp_sb")
                nc.vector.tensor_copy(dtp_sb, dT[:, 0:HEADS])
                recips = xsp.tile([128, HEADS], f32, name="recips")
                nc.vector.reciprocal(recips, dtp_sb)
                return x_sb, recips

            def proj_head(h, x_sb, recips, acc):
                """One head of output projection + fused normalize-accumulate."""
                ps = mm512.tile([128, DIM], f32, name="ps_mm512")
                nc.tensor.matmul(
                    ps,
                    x_sb[:, h, :],
                    wpT[h],
                    start=True,
                    stop=True,
                )
                nc.vector.scalar_tensor_tensor(
                    out=acc,
                    in0=ps,
                    scalar=recips[:, h:h + 1],
                    in1=(bias_sb if h == 0 else acc),
                    op0=MULT,
                    op1=ADD,
                )

            def kquad(kvt, kt, ko, ch):
                ps = mm512.tile([128, 512], f32, name="ps_mm512")
                for ci in range(4):
                    nc.tensor.matmul(
                        ps,
                        wkvT[ci][:, ko * 128:(ko + 1) * 128],
                        kvt[ci][:, ch * 512:(ch + 1) * 512],
                        start=(ci == 0),
                        stop=(ci == 3),
                    )
                nc.vector.tensor_copy(kt[ko][:, ch * 512:(ch + 1) * 512], ps)

            def vquad(kvt, vt, tt):
                ps = mm512.tile([128, 512], f32, name="ps_mm512")
                for ci in range(4):
                    nc.tensor.matmul(
                        ps,
                        kvt[ci][:, tt * 128:(tt + 1) * 128],
                        wkvT[ci][:, DIM:2 * DIM],
                        start=(ci == 0),
                        stop=(ci == 3),
                    )
                nc.scalar.copy(
                    vt[tt][:, :, 0:64], ps[:, :].rearrange("p (h d) -> p h d", h=HEADS)
                )
                nc.gpsimd.memset(vt[tt][:, :, 64:65], 1.0)

            def emit_av_pair(xaug, vt, pr, pt0, pt1):
                # One fully-serial 32-matmul accumulation chain per head:
                # interleaved open chains in one PSUM bank corrupt the
                # accumulator, so heads never interleave.
                for h, pt in ((2 * pr, pt0), (2 * pr + 1, pt1)):
                    for t in range(NT):
                        nc.tensor.matmul(
                            xaug[:, h, :],
                            vt[t][:, h, :],
                            pt[:, t * 128:(t + 1) * 128],
                            start=(t == 0),
                            stop=(t == NT - 1),
                            skip_group_check=True,
                        )

            steps = [b for _ in range(reps) for b in range(BL)]
            fetched = fetch(steps[0])
            pending = None  # (b, x_sb, recips) awaiting proj phase 2

            def fake_out(b):
                acc = xsp.tile([128, DIM], f32, name="acc")
                nc.vector.tensor_copy(acc, bias_sb)
                nc.sync.dma_start(out=out_d[b], in_=acc)

            for i, b in enumerate(steps):
                kvt, em_t = fetched
                # ---- A phase: kv projection (+ dribbled proj of prev batch)
                kt = [ktp.tile([128, N], bf16, name="k_t") for _ in range(4)]
                vt = [vp.tile([128, HEADS, 65], bf16, name="v_t") for _ in range(NT)]
                quads = [("k", ko, ch) for ko in range(4) for ch in range(N // 512)]
                quads += [("v", tt, 0) for tt in range(NT)]
                acc = None
                for qi, (kind, a0, a1) in enumerate(quads):
                    if pending is not None and qi < HEADS:
                        if qi == 0:
                            acc = xsp.tile([128, DIM], f32, name="acc")
                        proj_head(qi, pending[1], pending[2], acc)
                    if kind == "k":
                        kquad(kvt, kt, a0, a1)
                    else:
                        vquad(kvt, vt, a0)
                    if pending is not None and qi == HEADS:
                        nc.sync.dma_start(out=out_d[pending[0]], in_=acc)
                        pending = None

                if VARIANT == "kvonly":
                    fake_out(b)
                    if i + 1 < len(steps):
                        fetched = fetch(steps[i + 1])
                    continue

                # ---- B phase: attention, S^T orientation
                xaug = xaps.tile([65, HEADS, 128], f32, name="xaug")
                av_prev = None
                for pr in range(4):
                    pt0 = ptp.tile([128, N], bf16, name="pt0")
                    pt1 = ptp.tile([128, N], bf16, name="pt1")
                    for g in range(NG):
                        ps0 = qkps.tile([128, GW], f32, name="ps_s0")
                        ps1 = qkps.tile([128, GW], f32, name="ps_s1")
                        for j in range(GW // 128):
                            t = (GW // 128) * g + j
                            nc.tensor.matmul(
                                ps0[:, j * 128:(j + 1) * 128],
                                kt[pr][0:64, t * 128:(t + 1) * 128],
                                qhT[pr][0:64, b * QN:(b + 1) * QN],
                                start=True,
                                stop=True,
                                tile_position=(0, 0),
                            )
                            nc.tensor.matmul(
                                ps1[:, j * 128:(j + 1) * 128],
                                kt[pr][64:128, t * 128:(t + 1) * 128],
                                qhT[pr][64:128, b * QN:(b + 1) * QN],
                                start=True,
                                stop=True,
                                tile_position=(64, 0),
                            )
                        sl = slice(g * GW, (g + 1) * GW)
                        nc.scalar.activation(pt0[:, sl], ps0, EXP)
                        nc.scalar.activation(pt1[:, sl], ps1, EXP)
                        nc.vector.tensor_mul(pt0[:, sl], pt0[:, sl], em_t[:, sl])
                        nc.vector.tensor_mul(pt1[:, sl], pt1[:, sl], em_t[:, sl])
                        if pr == 1 and g == 0 and i + 1 < len(steps):
                            fetched = fetch(steps[i + 1])
                    if dbg and i == 0 and pr == 0:
                        nc.sync.dma_start(out=dbg_pt[0], in_=pt0)
                        nc.sync.dma_start(out=dbg_pt[1], in_=pt1)
                    if av_prev is not None and VARIANT != "noav":
                        emit_av_pair(xaug, vt, *av_prev)
                    av_prev = (pr, pt0, pt1)
                if VARIANT == "noav":
                    fake_out(b)
                    continue
                emit_av_pair(xaug, vt, *av_prev)
                if dbg and i == 0:
                    for t in range(4):
                        nc.sync.dma_start(out=dbg_kt[t], in_=kt[t])
                    for t in range(2):
                        nc.sync.dma_start(out=dbg_v[t], in_=vt[t])

                # ---- proj phase 1: denominators to q-partitions
                x_sb, recips = proj_denoms(xaug)
                if dbg and i == 0:
                    nc.sync.dma_start(out=dbg_x, in_=x_sb)
                    nc.sync.dma_start(out=dbg_r, in_=recips)
                pending = (b, x_sb, recips)

            # epilogue: flush last batch's projection
            if pending is not None:
                acc = xsp.tile([128, DIM], f32, name="acc")
                for h in range(HEADS):
                    proj_head(h, pending[1], pending[2], acc)
                nc.sync.dma_start(out=out_d[pending[0]], in_=acc)
    return nc


def build(reps=1):
    if reps not in _built:
        nc = bacc.Bacc(
            "TRN2", target_bir_lowering=False, debug=False, num_devices=NCORES
        )
        _emit(nc, reps)
        nc.compile()
        _built[reps] = nc
    return _built[reps]


def prep_inputs(q, kv, key_mask, Wq, Wkv, Wproj, bproj):
    """Host-side shard + layout prep. Returns per-core in_maps."""
    q = np.asarray(q, dtype=np.float32)
    kv = np.asarray(kv, dtype=np.float32)
    key_mask = np.asarray(key_mask, dtype=np.float32)
    wkvT = np.ascontiguousarray(np.asarray(Wkv, np.float32).T).astype(BF)
    wkvT = wkvT.reshape(4, 128, 2 * DIM)
    wqT = np.ascontiguousarray((np.asarray(Wq, np.float32) * SCALE).T).astype(BF)
    wqT = wqT.reshape(4, 128, DIM)
    wpT = np.ascontiguousarray(np.asarray(Wproj, np.float32).T).astype(BF)
    wpT = wpT.reshape(HEADS, 64, DIM)
    biasb = np.ascontiguousarray(
        np.broadcast_to(np.asarray(bproj, np.float32), (128, DIM))
    )

    kv_bf = kv.astype(BF)
    em = np.exp(key_mask).astype(BF)  # [B, QN, N]

    in_maps = []
    for c in range(NCORES):
        sl = slice(c * BL, (c + 1) * BL)
        kvT = np.ascontiguousarray(kv_bf[sl].transpose(0, 2, 1)).reshape(
            BL, 4, 128, N
        )
        q_loc = q[sl].astype(BF)  # [BL, QN, DIM]
        qT = np.ascontiguousarray(q_loc.transpose(2, 0, 1)).reshape(4, 128, BL * QN)
        # emT[b, n_in_tile, tt*128 + q] = em[b, q, tt*128 + n_in_tile]
        emT = em[sl].reshape(BL, QN, NT, 128).transpose(0, 3, 2, 1)
        emT = np.ascontiguousarray(emT).reshape(BL, 128, N)
        in_maps.append(
            {
                "kvT": kvT,
                "qT": qT,
                "emT": emT,
                "wkvT": wkvT,
                "wqT": wqT,
                "wpT": wpT,
                "biasb": biasb,
            }
        )
    return in_maps


class Runner:
    """Jitted SPMD executor with device-resident inputs for repeat timing."""

    def __init__(self, reps=1):
        import jax
        from concourse.bass2jax import (
            _bass_exec_p,
            install_neuronx_cc_hook,
            partition_id_tensor,
        )
        from jax.experimental.shard_map import shard_map
        from jax.sharding import Mesh, PartitionSpec

        self.jax = jax
        nc = build(reps)
        install_neuronx_cc_hook()
        pname = nc.partition_id_tensor.name if nc.partition_id_tensor else None
        in_names, out_names, out_avals = [], [], []
        for alloc in nc.m.functions[0].allocations:
            if not isinstance(alloc, mybir.MemoryLocationSet):
                continue
            name = alloc.memorylocations[0].name
            if alloc.kind == "ExternalInput":
                if name != pname:
                    in_names.append(name)
            elif alloc.kind == "ExternalOutput":
                out_names.append(name)
                out_avals.append(
                    jax.core.ShapedArray(
                        tuple(alloc.tensor_shape), mybir.dt.np(alloc.dtype)
                    )
                )
        self.in_names = list(in_names)
        self.out_names = out_names
        self.out_avals = out_avals
        n_params = len(in_names)
        all_names = in_names + out_names
        if pname is not None:
            all_names = all_names + [pname]
        donate = tuple(range(n_params, n_params + len(out_names)))

        def _body(*args):
            operands = list(args)
            if pname is not None:
                operands.append(partition_id_tensor())
            outs = _bass_exec_p.bind(
                *operands,
                out_avals=tuple(out_avals),
                in_names=tuple(all_names),
                out_names=tuple(out_names),
                lowering_input_output_aliases=(),
                sim_require_finite=True,
                sim_require_nnan=True,
                nc=nc,
            )
            return tuple(outs)

        devices = jax.devices()[:NCORES]
        self.mesh = Mesh(np.asarray(devices), ("core",))
        self.pspec = PartitionSpec("core")
        in_specs = (self.pspec,) * (n_params + len(out_names))
        out_specs = (self.pspec,) * len(out_names)
        self.fn = jax.jit(
            shard_map(
                _body,
                mesh=self.mesh,
                in_specs=in_specs,
                out_specs=out_specs,
                check_rep=False,
            ),
            donate_argnums=donate,
            keep_unused=True,
        )

    def put_inputs(self, in_maps):
        """Concat per-core inputs on axis 0 and move to devices (sharded)."""
        from jax.sharding import NamedSharding

        sh = NamedSharding(self.mesh, self.pspec)
        dev = []
        for name in self.in_names:
            cat = np.concatenate([m[name] for m in in_maps], axis=0)
            dev.append(self.jax.device_put(cat, sh))
        return dev

    def zeros(self):
        from jax.sharding import NamedSharding

        sh = NamedSharding(self.mesh, self.pspec)
        return [
            self.jax.device_put(
                np.zeros((NCORES * a.shape[0], *a.shape[1:]), a.dtype), sh
            )
            for a in self.out_avals
        ]

    def run(self, dev_inputs, zeros=None):
        if zeros is None:
            zeros = self.zeros()
        outs = self.fn(*dev_inputs, *zeros)
        self.jax.block_until_ready(outs)
        return outs


def get_runner(reps=1):
    if reps not in _runner:
        _runner[reps] = Runner(reps)
    return _runner[reps]


def kernel(q, kv, key_mask, Wq, Wkv, Wproj, bproj):
    r = get_runner()
    in_maps = prep_inputs(q, kv, key_mask, Wq, Wkv, Wproj, bproj)
    dev = r.put_inputs(in_maps)
    outs = r.run(dev)
    out = np.asarray(outs[0]).reshape(NCORES, BL, QN, DIM).reshape(B, QN, DIM)
    return out.astype(np.float32)
